# revision 1
# baseline (speedup 1.0000x reference)
"""Trainium2 Bass kernel: bidirectional transformer encoder block.

Data-parallel over batch: B=8 samples -> 8 NeuronCores, one sample each.
All compute per core is in "T layout" (features on partitions, tokens free).

fp8 (e4m3) DoubleRow matmuls throughout (157 TF/s): operand pairs over
contraction 128-tiles are interleaved along the free axis of both operands
([p, 2, n] access patterns on [P, 2*n] tiles).

Precision scheme (measured absmax ~0.03 @ scale 9.07, gate is 0.18):
 - attention (QKV, scores, PV, proj): naive fp8; weights prescaled x32 so
   e4m3 stays in normal range; scale unwound in the exp() activation scale
   (1/(32^2*sqrt(C))) and the proj eviction (1/32^2). Attention output is
   ~1% of h, so fp8 noise there is diluted ~50x.
 - FFN: 3-slot compensated fp8: u*WS = hn8@W8 + hnlo@W8 + hn8@W8lo, with
   W8 = fp8(W*WS), W8lo = fp8(W*WS - W8) (host-side), hnlo = fp8(hn - hn8)
   (one DVE op per tile). Restores ~bf16 accuracy at 1.5x bf16 matmul
   throughput.
 - residuals (xd, h) fp32; hn/acc bf16; LN stats fp32 rows.

Z (softmax denominator) is fused into the PV matmul as a 65th ones-column
per head in the V tiles; 1/Z is partition-broadcast with a tiny ones-matmul
into PSUM instead of the DRAM round trip.

"""

import numpy as np
import ml_dtypes

import concourse.bass as bass
import concourse.mybir as mybir
import concourse.tile as tile
from concourse import bacc
from concourse.bass_utils import run_bass_kernel_spmd

P = 128
T = 1024
C = 1024
H = 16
HS = 64
C2 = 2 * C
F = 8 * C
NT = C // P      # 8  c-tiles
NP = NT // 2     # 4  c-tile pairs
NT2 = C2 // P    # 16
NFT = F // P     # 64 f-tiles
SEG = 512
NSEG = T // SEG  # 2
EPS = 1e-5
WS = 32.0        # fp8 weight prescale
F32 = mybir.dt.float32
BF16 = mybir.dt.bfloat16
F8 = mybir.dt.float8e4
AF = mybir.ActivationFunctionType
OP = mybir.AluOpType
DR = mybir.MatmulPerfMode.DoubleRow
N_CORES = 8

ESC = 1.0 / (WS * WS * 32.0)   # exp scale: 1/(WS^2 * sqrt(C))
PSC = 1.0 / (WS * WS)          # proj eviction scale
USC = 1.0 / WS                 # FFN eviction scale


def build_nc():
    nc = bacc.Bacc(None, target_bir_lowering=False, debug=False)

    # ---- DRAM I/O ----
    xT = nc.dram_tensor("xT", [C, T], F32, kind="ExternalInput")
    xfT = nc.dram_tensor("xfT", [C, T], F32, kind="ExternalInput")
    w8 = {}
    for d in ("f", "b"):
        for w in ("wq", "wk", "wv", "wp"):
            w8[f"{w}_{d}"] = nc.dram_tensor(
                f"{w}8_{d}", [NP * P, 2 * C], F8, kind="ExternalInput")
    w1a = nc.dram_tensor("w1a", [(C2 // 256) * P, 2 * F], F8,
                         kind="ExternalInput")
    w1c = nc.dram_tensor("w1c", [(C2 // 256) * P, 2 * F], F8,
                         kind="ExternalInput")
    w2a = nc.dram_tensor("w2a", [(F // 256) * P, 2 * C2], F8,
                         kind="ExternalInput")
    w2c = nc.dram_tensor("w2c", [(F // 256) * P, 2 * C2], F8,
                         kind="ExternalInput")
    vec_c = {}
    for nm in ("g_f", "b_f", "g_b", "b_b", "bq_f", "bk_f", "bq_b", "bk_b",
               "bp_f", "bp_b"):
        vec_c[nm] = nc.dram_tensor(nm, [C], F32, kind="ExternalInput")
    g2v = nc.dram_tensor("g2v", [C2], F32, kind="ExternalInput")
    b2lnv = nc.dram_tensor("b2lnv", [C2], F32, kind="ExternalInput")
    b1v = nc.dram_tensor("b1v", [F], F32, kind="ExternalInput")
    b2v = nc.dram_tensor("b2v", [C2], F32, kind="ExternalInput")
    ln1rows = nc.dram_tensor("ln1rows", [2, T], F32, kind="ExternalInput")

    outT = nc.dram_tensor("outT", [C2, T], F32, kind="ExternalOutput")

    # DRAM scratch
    hspill = nc.dram_tensor("hspill", [C2, T], F32)
    rows_dram = nc.dram_tensor("rows_dram", [2, T], F32)

    with tile.TileContext(nc) as tc:
        with (
            nc.allow_low_precision(
                reason="fp8 kernel: quantization error is compensated by "
                       "design (hi/lo slots); residual paths stay fp32"),
            tc.tile_pool(name="sb", bufs=1) as sb,
            tc.tile_pool(name="ps", bufs=2, space="PSUM") as ps,
        ):
            def psB(nm, grp="a"):
                return ps.tile([P, 2 * SEG], F32, name=nm, tag=f"ps{grp}")

            # ---- constants / vectors ----
            ones16 = sb.tile([P, 1], BF16, name="ones16", tag="ones16")
            nc.gpsimd.memset(ones16[:], 1.0)
            onesrow = sb.tile([P, HS], BF16, name="onesrow", tag="onesrow")
            nc.gpsimd.memset(onesrow[:], 1.0)
            zero_col = sb.tile([P, 1], F32, name="zero_col", tag="zero_col")
            nc.gpsimd.memset(zero_col[:], 0.0)

            def load_vec(handle, n_tiles, nm):
                t_ = sb.tile([P, n_tiles], F32, name=f"c_{nm}", tag=f"c_{nm}")
                nc.sync.dma_start(
                    t_[:], handle[:].rearrange("(a p) -> p a", p=P))
                return t_

            cols = {nm: load_vec(h_, C // P, nm) for nm, h_ in vec_c.items()}
            g2c = load_vec(g2v, NT2, "g2")
            b2lnc = load_vec(b2lnv, NT2, "b2ln")
            b1c = load_vec(b1v, NFT, "b1")
            b2c = load_vec(b2v, NT2, "b2")

            # stat rows (partition 0)
            rowA = sb.tile([1, T], F32, name="rowA", tag="rowA")
            rowB = sb.tile([1, T], F32, name="rowB", tag="rowB")
            rowC = sb.tile([1, T], F32, name="rowC", tag="rowC")

            def ln_stats(stream_src, n_ptiles, denom):
                """Column stats via bf16 ones-matmuls; leaves rsig in rowC,
                -mu*rsig in rowA."""
                rowD = sb.tile([1, T], F32, name="rowD", tag="zrow")
                ps_mu = psB("psmu", "a")
                ps_ms = psB("psms", "b")
                for i in range(n_ptiles):
                    xt = sb.tile([P, T], F32, name=f"st_x{i}", tag="xts",
                                 bufs=2)
                    nc.scalar.dma_start(xt[:], stream_src(i))
                    x16 = sb.tile([P, T], BF16, name=f"st_h{i}", tag="hio",
                                  bufs=2)
                    nc.vector.tensor_copy(x16[:], xt[:])
                    sq = sb.tile([P, T], BF16, name=f"st_sq{i}", tag="tpe",
                                 bufs=2)
                    nc.scalar.activation(sq[:], xt[:], AF.Square,
                                         bias=zero_col[:])
                    for s in range(NSEG):
                        sl = slice(s * SEG, (s + 1) * SEG)
                        nc.tensor.matmul(
                            ps_mu[0:1, sl], ones16[:], x16[:, sl],
                            start=(i == 0), stop=(i == n_ptiles - 1))
                        nc.tensor.matmul(
                            ps_ms[0:1, sl], ones16[:], sq[:, sl],
                            start=(i == 0), stop=(i == n_ptiles - 1))
                nc.vector.tensor_scalar(rowA[0:1, :], ps_mu[0:1, :],
                                        1.0 / denom, None, OP.mult)
                nc.vector.tensor_scalar(rowB[0:1, :], ps_ms[0:1, :],
                                        1.0 / denom, None, OP.mult)
                nc.vector.tensor_mul(rowC[0:1, :], rowA[0:1, :], rowA[0:1, :])
                nc.vector.scalar_tensor_tensor(
                    rowB[0:1, :], rowC[0:1, :], -1.0, rowB[0:1, :],
                    OP.mult, OP.add)
                nc.vector.tensor_scalar(rowB[0:1, :], rowB[0:1, :], EPS, None,
                                        OP.add)
                nc.scalar.activation(rowC[0:1, :], rowB[0:1, :], AF.Sqrt,
                                     bias=zero_col[0:1, :])
                nc.vector.reciprocal(rowC[0:1, :], rowC[0:1, :])
                nc.vector.tensor_mul(rowD[0:1, :], rowC[0:1, :], rowC[0:1, :])
                nc.vector.tensor_mul(rowD[0:1, :], rowD[0:1, :], rowB[0:1, :])
                nc.vector.tensor_scalar(rowD[0:1, :], rowD[0:1, :], -0.5, 1.5,
                                        OP.mult, OP.add)
                nc.vector.tensor_mul(rowC[0:1, :], rowC[0:1, :], rowD[0:1, :])
                nc.vector.scalar_tensor_tensor(
                    rowA[0:1, :], rowA[0:1, :], -1.0, rowC[0:1, :],
                    OP.mult, OP.mult)

            # ==== LN1 stats from host (exact fp32) ====
            rsbc = sb.tile([P, T], F32, name="rsbc0", tag="rsbc")
            nmbc = sb.tile([P, T], F32, name="nmbc0", tag="nmbc")
            nc.sync.dma_start(rsbc[:], ln1rows[0:1, :].to_broadcast((P, T)))
            nc.sync.dma_start(nmbc[:], ln1rows[1:2, :].to_broadcast((P, T)))

            def pair2(ap):
                return ap.rearrange("p (i n) -> p i n", i=2)

            # ======================= attention dirs =======================
            xd8p_b_saved = None
            for dix, d in enumerate(("f", "b")):
                xsrc = xT if d == "f" else xfT

                # fp8 weight pair-tiles stream through 4 shared slots
                def load_w(w):
                    ts_ = []
                    for jp in range(NP):
                        t_ = sb.tile([P, 2 * C], F8, name=f"{w}8{d}{jp}",
                                     tag=f"wt{jp}")
                        (nc.sync if jp % 2 == 0 else nc.scalar).dma_start(
                            t_[:], w8[f"{w}_{d}"][jp * P:(jp + 1) * P, :])
                        ts_.append(t_)
                    return ts_

                # ---- LN1 apply -> xd (f32) + xd8p (fp8 pairs) ----
                xd = [sb.tile([P, T], F32, name=f"xd_{d}{i}", tag=f"xd{i}")
                      for i in range(NT)]
                if dix == 0:
                    xd8p = [sb.tile([P, 2 * T], F8, name=f"xp{d}{j}",
                                    tag=f"xp{j}") for j in range(NP)]
                else:
                    xd8p = xd8p_b_saved
                gcol = cols[f"g_{d}"]
                bcol = cols[f"b_{d}"]
                for i in range(NT):
                    xs = sb.tile([P, T], F32, name=f"ln_x_{d}{i}", tag="xts",
                                 bufs=2)
                    (nc.sync if i % 2 == 0 else nc.scalar).dma_start(
                        xs[:], xsrc[i * P:(i + 1) * P, :])
                    t0 = sb.tile([P, T], F32, name=f"ln_t_{d}{i}", tag="lnt",
                                 bufs=2)
                    nc.gpsimd.tensor_mul(t0[:], xs[:], rsbc[:])
                    nc.vector.scalar_tensor_tensor(
                        t0[:], t0[:], 0.0, nmbc[:], OP.bypass, OP.add)
                    nc.gpsimd.tensor_scalar(
                        xd[i][:], t0[:], gcol[:, i:i + 1],
                        bcol[:, i:i + 1], OP.mult, OP.add)
                    if dix == 0:
                        nc.gpsimd.tensor_copy(
                            xd8p[i // 2][:, (i % 2) * T:(i % 2 + 1) * T],
                            xd[i][:])

                # ---- Q/K projections (fp8 DoubleRow) -> qt/kt fp8 ----
                qt = [sb.tile([P, T], F8, name=f"qt{d}{i}", tag=f"qt{i}")
                      for i in range(NT)]
                kt = [sb.tile([P, T], F8, name=f"kt{d}{i}", tag=f"kt{i}")
                      for i in range(NT)]
                for (wname, dst, bias) in (("wq", qt, cols[f"bq_{d}"]),
                                           ("wk", kt, cols[f"bk_{d}"])):
                    wts = load_w(wname)
                    for co in range(NT):
                        for s in range(NSEG):
                            sl = slice(s * SEG, (s + 1) * SEG)
                            psq = psB(f"psq{wname}{co}{s}",
                                      "ab"[s % 2])[:, 0:SEG]
                            for jp in range(NP):
                                nc.tensor.matmul(
                                    psq[:, :],
                                    pair2(wts[jp][:, :])[
                                        :, :, co * P:(co + 1) * P],
                                    pair2(xd8p[jp][:, :])[:, :, sl],
                                    start=(jp == 0), stop=(jp == NP - 1),
                                    perf_mode=DR)
                            nc.vector.tensor_scalar(
                                dst[co][:, sl], psq[:, :],
                                bias[:, co:co + 1], None, OP.add)

                # ---- V (token-major, fp8 DR) -> v_pair with ones cols ----
                VW = 2 * (H * (HS + 1))   # 2 * 1040
                vp = [sb.tile([P, VW], F8, name=f"vp{d}{j}", tag=f"vp{j}")
                      for j in range(NP)]
                for j in range(NP):
                    nc.gpsimd.memset(
                        vp[j][:, :].rearrange(
                            "p (i h c) -> p i h c", i=2, h=H)[:, :, :, HS:],
                        1.0)
                wts = load_w("wv")
                for s in range(NSEG):
                    for t_ in range(8):
                        psv = psB(f"psv{s}{t_}", "ab"[t_ % 2])[:, 0:SEG]
                        for jp in range(NP):
                            nc.tensor.matmul(
                                psv[:, :],
                                pair2(xd8p[jp][:, :])[
                                    :, :, t_ * P:(t_ + 1) * P],
                                pair2(wts[jp][:, :])[
                                    :, :, s * SEG:(s + 1) * SEG],
                                start=(jp == 0), stop=(jp == NP - 1),
                                perf_mode=DR)
                        vstage = sb.tile([P, SEG], F8, name=f"vs{s}{t_}",
                                         tag="vstage", bufs=3)
                        nc.vector.tensor_copy(vstage[:], psv[:, :])
                        for b in range(8):
                            hh = s * 8 + b
                            nc.gpsimd.tensor_copy(
                                vp[t_ // 2][:, (t_ % 2) * (VW // 2)
                                            + hh * (HS + 1):
                                            (t_ % 2) * (VW // 2)
                                            + hh * (HS + 1) + HS],
                                vstage[:, b * HS:(b + 1) * HS])

                if dix == 0:
                    # dir-b fp8 LN output early (overlaps dir-f attention)
                    xd8p_b_saved = [
                        sb.tile([P, 2 * T], F8, name=f"xpb{j}",
                                tag=f"xpb{j}") for j in range(NP)]
                    gb, bb = cols["g_b"], cols["b_b"]
                    for i in range(NT):
                        xs = sb.tile([P, T], F32, name=f"lnb_x{i}", tag="xts",
                                     bufs=2)
                        nc.sync.dma_start(xs[:], xfT[i * P:(i + 1) * P, :])
                        t0 = sb.tile([P, T], F32, name=f"lnb_t{i}", tag="lnt",
                                     bufs=2)
                        nc.gpsimd.tensor_mul(t0[:], xs[:], rsbc[:])
                        nc.vector.scalar_tensor_tensor(
                            t0[:], t0[:], 0.0, nmbc[:], OP.bypass, OP.add)
                        nc.gpsimd.tensor_scalar(
                            t0[:], t0[:], gb[:, i:i + 1],
                            bb[:, i:i + 1], OP.mult, OP.add)
                        nc.gpsimd.tensor_copy(
                            xd8p_b_saved[i // 2][:, (i % 2) * T:
                                                 (i % 2 + 1) * T], t0[:])

                # ---- attention: scores fp8, exp, PV fp8-DR with Z row ----
                otp = [sb.tile([P, 2 * T], F8, name=f"ot{d}{j}",
                               tag=f"op{j}") for j in range(NP)]
                # Z normalization is software-pipelined one (h,s) stage
                # behind PV so the PE never waits on the DVE recip.
                pend = []

                def flush_z():
                    if not pend:
                        return
                    psuz2, dsl, oj2, ob2 = pend.pop(0)
                    zsb = sb.tile([P, SEG], BF16, name=f"zs{d}{len(pend)}",
                                  tag="zsb", bufs=3)
                    nc.vector.reciprocal(zsb[HS:HS + 1, :],
                                         psuz2[HS:HS + 1, 0:SEG])
                    nc.tensor.matmul(psuz2[0:HS, SEG:2 * SEG],
                                     onesrow[HS:HS + 1, :],
                                     zsb[HS:HS + 1, :])
                    zc = sb.tile([P, SEG], BF16, name=f"zc{d}{len(pend)}",
                                 tag="zc", bufs=3)
                    nc.vector.tensor_copy(zc[0:HS, :],
                                          psuz2[0:HS, SEG:2 * SEG])
                    nc.vector.tensor_mul(otp[oj2][ob2:ob2 + HS, dsl],
                                         psuz2[0:HS, 0:SEG], zc[0:HS, :])

                for h in range(H):
                    pt, off = h // 2, (h % 2) * HS
                    ob = (h % 2) * HS          # row offset in ot pair tile
                    oj, oi = (h // 2) // 2, (h // 2) % 2
                    for s in range(NSEG):
                        sl = slice(s * SEG, (s + 1) * SEG)
                        psuz = psB(f"psu{h}{s}", "a")

                        def sc_exp(jp):
                            est = sb.tile([P, 2 * SEG], F8,
                                          name=f"es{h}{s}{jp}", tag="est",
                                          bufs=4)
                            pss = psB(f"pss{h}{s}{jp}", "b")
                            for i in range(2):
                                t2 = 2 * jp + i
                                nc.tensor.matmul(
                                    pss[:, i * SEG:(i + 1) * SEG],
                                    kt[pt][off:off + HS,
                                           t2 * P:(t2 + 1) * P],
                                    qt[pt][off:off + HS, sl])
                            nc.scalar.activation(
                                est[:, :], pss[:, :], AF.Exp,
                                bias=zero_col[:], scale=ESC)
                            return est

                        # scores/exp run two stages ahead of PV so the
                        # Act exp stream never waits on the PV chain
                        ests = [sc_exp(0), sc_exp(1)]
                        for jp in range(NP):
                            if jp + 2 < NP:
                                ests.append(sc_exp(jp + 2))
                            nc.tensor.matmul(
                                psuz[0:HS + 1, 0:SEG],
                                pair2(vp[jp][:, :])[
                                    :, :, h * (HS + 1):(h + 1) * (HS + 1)],
                                pair2(ests[jp][:, :]),
                                start=(jp == 0), stop=(jp == NP - 1),
                                perf_mode=DR)
                        dsl = slice(oi * T + s * SEG,
                                    oi * T + (s + 1) * SEG)
                        pend.append((psuz, dsl, oj, ob))
                        if len(pend) > 1:
                            flush_z()
                flush_z()

                # ---- output projection (fp8 DR) + residual -> hspill ----
                dbase = 0 if d == "f" else NT
                bpcol = cols[f"bp_{d}"]
                wts = load_w("wp")
                for co in range(NT):
                    for s in range(NSEG):
                        sl = slice(s * SEG, (s + 1) * SEG)
                        psy = psB(f"psy{co}{s}", "ab"[s % 2])[:, 0:SEG]
                        for jo in range(NP):
                            nc.tensor.matmul(
                                psy[:, :],
                                pair2(wts[jo][:, :])[
                                    :, :, co * P:(co + 1) * P],
                                pair2(otp[jo][:, :])[:, :, sl],
                                start=(jo == 0), stop=(jo == NP - 1),
                                perf_mode=DR)
                        tp = sb.tile([P, SEG], F32, name=f"tp{d}{co}{s}",
                                     tag="tpe", bufs=2)
                        nc.scalar.activation(tp[:], psy[:, :], AF.Copy,
                                             bias=0.0, scale=PSC)
                        nc.gpsimd.tensor_scalar(tp[:], tp[:],
                                                bpcol[:, co:co + 1], None,
                                                OP.add)
                        hio = sb.tile([P, SEG], F32, name=f"h{d}{co}{s}",
                                      tag="hio", bufs=2)
                        nc.vector.tensor_add(hio[:], tp[:], xd[co][:, sl])
                        nc.sync.dma_start(
                            hspill[(dbase + co) * P:(dbase + co + 1) * P, sl],
                            hio[:])

            # =========== LN2 over concat features ===========
            ln_stats(lambda i: hspill[i * P:(i + 1) * P, :], NT2, float(C2))
            nc.sync.dma_start(rows_dram[0:1, :], rowC[0:1, :])
            nc.sync.dma_start(rows_dram[1:2, :], rowA[0:1, :])
            rs2 = sb.tile([P, T], F32, name="rs2", tag="rsbc")
            nm2 = sb.tile([P, T], F32, name="nm2", tag="nmbc")
            nc.sync.dma_start(rs2[:], rows_dram[0:1, :].to_broadcast((P, T)))
            nc.sync.dma_start(nm2[:], rows_dram[1:2, :].to_broadcast((P, T)))

            hn16 = []
            for i in range(NT2):
                hn16.append(sb.tile([P, T], BF16, name=f"hn{i}",
                                    tag=(f"xd{i}" if i < NT
                                         else f"qt{i - NT}")))
            hn8p = [sb.tile([P, 2 * T], F8, name=f"hn8p{j}",
                            tag=(f"xp{j}" if j < NP else f"xpb{j - NP}"))
                    for j in range(NT2 // 2)]
            hnlop = [sb.tile([P, 2 * T], F8, name=f"hnlop{j}",
                             tag=(f"vp{j}" if j < NP else f"op{j - NP}"))
                     for j in range(NT2 // 2)]
            for i in range(NT2):
                hs = sb.tile([P, T], F32, name=f"l2x{i}", tag="xts", bufs=2)
                eng = nc.sync if i % 2 == 0 else nc.scalar
                eng.dma_start(hs[:], hspill[i * P:(i + 1) * P, :])
                t0 = sb.tile([P, T], F32, name=f"l2t{i}", tag="lnt", bufs=2)
                nc.gpsimd.tensor_mul(t0[:], hs[:], rs2[:])
                nc.vector.scalar_tensor_tensor(
                    t0[:], t0[:], 0.0, nm2[:], OP.bypass, OP.add)
                nc.gpsimd.tensor_scalar(
                    hn16[i][:], t0[:], g2c[:, i:i + 1], b2lnc[:, i:i + 1],
                    OP.mult, OP.add)
                hsl = slice((i % 2) * T, (i % 2 + 1) * T)
                nc.gpsimd.tensor_copy(hn8p[i // 2][:, hsl], hn16[i][:])
                nc.vector.scalar_tensor_tensor(
                    hnlop[i // 2][:, hsl], hn8p[i // 2][:, hsl], -1.0,
                    hn16[i][:], OP.mult, OP.add)

            # =========== FFN (3-slot compensated fp8 DR) ===========
            # th-outer: W1 uses 2 [P,1024] psum tiles (j-pairs), W2 uses 2,
            # so W1 of chunk c+1 pipelines with W2 of chunk c (4+4 banks).
            acc_tags = ([f"wt{j}" for j in range(NP)]
                        + [f"kt{i}" for i in range(NT)]
                        + [f"acc{n}" for n in range(20)])
            acc = [[sb.tile([P, SEG], BF16, name=f"acc{c}_{th}",
                            tag=acc_tags[c * NSEG + th])
                    for th in range(NSEG)] for c in range(NT2)]
            for th in range(NSEG):
                rsl = slice(th * SEG, (th + 1) * SEG)
                for chunk in range(8):
                    u8p = [sb.tile([P, 2 * SEG], F8,
                                   name=f"u8_{th}{chunk}{m}",
                                   tag=f"u8_{m}", bufs=2) for m in range(4)]
                    ulop = [sb.tile([P, 2 * SEG], F8,
                                    name=f"ul_{th}{chunk}{m}",
                                    tag=f"ul_{m}", bufs=1) for m in range(4)]
                    for jq in range(2):
                        psw = [psB(f"psw{th}{chunk}{jq}{jj}", "a")
                               for jj in range(2)]
                        for jp in range(NT2 // 2):
                            base = chunk * 1024 + jq * 512
                            w1at = sb.tile([P, 1024], F8,
                                           name=f"w1a{th}{chunk}{jq}{jp}",
                                           tag="w1s", bufs=3)
                            nc.sync.dma_start(
                                w1at[:],
                                w1a[jp * P:(jp + 1) * P, :].rearrange(
                                    "p (i f) -> p i f", i=2)[
                                    :, :, base:base + 512])
                            w1ct = sb.tile([P, 1024], F8,
                                           name=f"w1c{th}{chunk}{jq}{jp}",
                                           tag="w1cs", bufs=3)
                            nc.gpsimd.dma_start(
                                w1ct[:],
                                w1c[jp * P:(jp + 1) * P, :].rearrange(
                                    "p (i f) -> p i f", i=2)[
                                    :, :, base:base + 512])
                            for j in range(4):
                                lha = pair2(w1at[:, :])[
                                    :, :, j * P:(j + 1) * P]
                                lhc = pair2(w1ct[:, :])[
                                    :, :, j * P:(j + 1) * P]
                                out = psw[j // 2][:, (j % 2) * SEG:
                                                  (j % 2 + 1) * SEG]
                                ra = pair2(hn8p[jp][:, :])[:, :, rsl]
                                rl = pair2(hnlop[jp][:, :])[:, :, rsl]
                                nc.tensor.matmul(
                                    out, lha, ra, start=(jp == 0),
                                    stop=False, perf_mode=DR)
                                nc.tensor.matmul(
                                    out, lha, rl, start=False, stop=False,
                                    perf_mode=DR)
                                nc.tensor.matmul(
                                    out, lhc, ra, start=False,
                                    stop=(jp == NT2 // 2 - 1), perf_mode=DR)
                        for j in range(4):
                            jf = jq * 4 + j
                            fglob = chunk * 8 + jf
                            m, half = jf // 2, jf % 2
                            usl = slice(half * SEG, (half + 1) * SEG)
                            pw = psw[j // 2][:, (j % 2) * SEG:
                                             (j % 2 + 1) * SEG]
                            nc.scalar.activation(
                                u8p[m][:, usl], pw, AF.Relu,
                                bias=b1c[:, fglob:fglob + 1], scale=USC)
                            ub = sb.tile([P, SEG], BF16,
                                         name=f"ub{th}{chunk}{jf}",
                                         tag="ubf", bufs=4)
                            nc.scalar.activation(
                                ub[:], pw, AF.Relu,
                                bias=b1c[:, fglob:fglob + 1], scale=USC)
                            nc.vector.scalar_tensor_tensor(
                                ulop[m][:, usl], u8p[m][:, usl],
                                -1.0, ub[:], OP.mult, OP.add)
                    # --- W2 into acc ---
                    for c2h in range(2):
                        for c2q in range(2):
                            cbase = c2h * 1024 + c2q * 512
                            psf = [psB(f"psf{th}{chunk}{c2h}{c2q}{jj}", "b")
                                   for jj in range(2)]
                            for g in range(4):
                                jg = chunk * 4 + g
                                w2at = sb.tile(
                                    [P, 1024], F8,
                                    name=f"w2a{th}{chunk}{cbase}{g}",
                                    tag="w2s", bufs=3)
                                nc.scalar.dma_start(
                                    w2at[:],
                                    w2a[jg * P:(jg + 1) * P, :].rearrange(
                                        "p (i c) -> p i c", i=2)[
                                        :, :, cbase:cbase + 512])
                                w2ct = sb.tile(
                                    [P, 1024], F8,
                                    name=f"w2c{th}{chunk}{cbase}{g}",
                                    tag="w2cs", bufs=3)
                                nc.sync.dma_start(
                                    w2ct[:],
                                    w2c[jg * P:(jg + 1) * P, :].rearrange(
                                        "p (i c) -> p i c", i=2)[
                                        :, :, cbase:cbase + 512])
                                for j2 in range(4):
                                    lha = pair2(w2at[:, :])[
                                        :, :, j2 * P:(j2 + 1) * P]
                                    lhc = pair2(w2ct[:, :])[
                                        :, :, j2 * P:(j2 + 1) * P]
                                    out = psf[j2 // 2][:, (j2 % 2) * SEG:
                                                       (j2 % 2 + 1) * SEG]
                                    ra = pair2(u8p[g][:, :])
                                    rl = pair2(ulop[g][:, :])
                                    nc.tensor.matmul(
                                        out, lha, ra, start=(g == 0),
                                        stop=False, perf_mode=DR)
                                    nc.tensor.matmul(
                                        out, lha, rl, start=False,
                                        stop=False, perf_mode=DR)
                                    nc.tensor.matmul(
                                        out, lhc, ra, start=False,
                                        stop=(g == 3), perf_mode=DR)
                            for j2 in range(4):
                                c2g = c2h * 8 + c2q * 4 + j2
                                pf = psf[j2 // 2][:, (j2 % 2) * SEG:
                                                  (j2 % 2 + 1) * SEG]
                                if chunk == 0:
                                    nc.vector.tensor_scalar(
                                        acc[c2g][th][:], pf, USC, None,
                                        OP.mult)
                                else:
                                    nc.vector.scalar_tensor_tensor(
                                        acc[c2g][th][:], pf, USC,
                                        acc[c2g][th][:], OP.mult, OP.add)
                                if chunk == 7:
                                    tsl = slice(th * SEG, (th + 1) * SEG)
                                    te = sb.tile([P, SEG], F32,
                                                 name=f"te{c2g}{th}",
                                                 tag="tpe", bufs=2)
                                    nc.gpsimd.tensor_scalar(
                                        te[:], acc[c2g][th][:],
                                        b2c[:, c2g:c2g + 1], None, OP.add)
                                    ob = sb.tile([P, SEG], F32,
                                                 name=f"ob{c2g}{th}",
                                                 tag="hio", bufs=2)
                                    nc.vector.tensor_add(
                                        ob[:], te[:], hn16[c2g][:, tsl])
                                    nc.sync.dma_start(
                                        outT[c2g * P:(c2g + 1) * P, tsl],
                                        ob[:])

    nc.compile()
    return nc


def _pack_pairs(w8_2d):
    """[K, M] fp8 -> [K//2, 2*M] with 128-row pair tiles interleaved along
    the free axis: out[j*128+p, i*M+m] = w[(2j+i)*128+p, m]."""
    K, M = w8_2d.shape
    return np.ascontiguousarray(
        w8_2d.reshape(K // 256, 2, 128, M).transpose(0, 2, 1, 3)
        .reshape(K // 2, 2 * M))


def _prep_inputs(inputs):
    f32 = np.float32
    f8 = ml_dtypes.float8_e4m3
    ws = f32(WS)

    def flat_qkv(w):  # [H, C, HS] -> [C, H*HS]
        return np.ascontiguousarray(
            np.transpose(np.asarray(w, f32), (1, 0, 2)).reshape(C, C))

    m = {}
    for d in ("f", "b"):
        m[f"wq8_{d}"] = _pack_pairs((flat_qkv(inputs[f"{d}_Wq"]) * ws)
                                    .astype(f8))
        m[f"wk8_{d}"] = _pack_pairs((flat_qkv(inputs[f"{d}_Wk"]) * ws)
                                    .astype(f8))
        m[f"wv8_{d}"] = _pack_pairs((flat_qkv(inputs[f"{d}_Wv"]) * ws)
                                    .astype(f8))
        wp_ = np.asarray(inputs[f"{d}_Wp"], f32)
        m[f"wp8_{d}"] = _pack_pairs((wp_ * ws).astype(f8))
        m[f"bq_{d}"] = np.asarray(inputs[f"{d}_bq"], f32).reshape(C) * ws
        m[f"bk_{d}"] = np.asarray(inputs[f"{d}_bk"], f32).reshape(C) * ws
        bv_flat = np.asarray(inputs[f"{d}_bv"], f32).reshape(C)
        m[f"bp_{d}"] = (np.asarray(inputs[f"{d}_bp"], f32)
                        + bv_flat @ wp_).astype(f32)
        m[f"g_{d}"] = np.asarray(inputs[f"{d}_ln_g"], f32)
        m[f"b_{d}"] = np.asarray(inputs[f"{d}_ln_b"], f32)
    for nm, w in (("w1", inputs["ffn_W1"]), ("w2", inputs["ffn_W2"])):
        wsF = np.asarray(w, f32) * ws
        a8 = wsF.astype(f8)
        c8 = (wsF - a8.astype(f32)).astype(f8)
        m[f"{nm}a"] = _pack_pairs(a8)
        m[f"{nm}c"] = _pack_pairs(c8)
    m["b1v"] = np.asarray(inputs["ffn_b1"], f32)
    m["b2v"] = np.asarray(inputs["ffn_b2"], f32)
    m["g2v"] = np.asarray(inputs["ln2_g"], f32)
    m["b2lnv"] = np.asarray(inputs["ln2_b"], f32)
    return m


_NC_CACHE = {}


def get_nc():
    if "nc" not in _NC_CACHE:
        _NC_CACHE["nc"] = build_nc()
    return _NC_CACHE["nc"]


def make_in_map(shared, x, b):
    im = dict(shared)
    im["xT"] = np.ascontiguousarray(x[b].T)
    im["xfT"] = np.ascontiguousarray(x[b][:, ::-1].T)
    xb64 = x[b].astype(np.float64)
    mu = xb64.mean(-1)
    rsig = 1.0 / np.sqrt(xb64.var(-1) + EPS)
    im["ln1rows"] = np.ascontiguousarray(
        np.stack([rsig, -mu * rsig]).astype(np.float32))
    return im


def kernel(**inputs):
    nc = get_nc()
    shared = _prep_inputs(inputs)
    x = np.asarray(inputs["x"], np.float32)
    in_maps = [make_in_map(shared, x, b) for b in range(N_CORES)]
    res = run_bass_kernel_spmd(nc, in_maps, core_ids=list(range(N_CORES)))
    out = np.stack([np.ascontiguousarray(r["outT"].T)
                    for r in res.results], axis=0)
    return out.astype(np.float32)

